# revision 44
# baseline (speedup 1.0000x reference)
"""ParallelHyenaOperator Trainium2 kernel.

out = (irfft(rfft(u,2L) * rfft(k,2L))[:L] + u*d_bias) * x1,  u = x2*v, k = h*decay

Sharding: D=768 channels split across 8 cores (96/core), no collectives.
Per core, channels are paired (c, c+48) and stacked in SBUF partitions
(c -> rows 0:64, c+48 -> rows 64:128), 8 pairs per slab, 6 slabs.
Each 16384-point FFT is a two-stage radix-128 factorization on the tensor
engine; both batches are packed as one complex series (z = u_b0 + i*u_b1).
Stage-1 matmuls take the stacked pair as the stationary operand against
block-diagonal DFT weights, producing both channels in one PSUM bank; the
final inverse stage writes the high channel to PSUM partitions 64:127
(PE tile_position col=64), so pre/post gating runs at full 128-partition
width. Twiddle/product stages run in bf16 on DVE (spectral-product
multiplies on GpSimd), double-wide over two pairs per op ([128,1024]) to
amortize per-op overhead; PSUM evacuations run on the scalar engine.
Inputs stream as whole slabs (4 dma_starts per tensor per slab); x1, h,
and decay are loaded as bf16 via gpsimd casting DMAs.

Measured on TRN2: ~359 us device exec (from 79.96 ms staged baseline);
rel err vs fp64 reference ~5.8e-3 (absmax-normalized), gate 2e-2.
"""

import math
import numpy as np
import ml_dtypes

B, D, L = 2, 768, 8192
NCORES = 8
DPC = D // NCORES          # 96 channels per core
HALF = DPC // 2            # 48; pairing (c, c+48)
SLABP = 8                  # pairs per slab
NSLAB = HALF // SLABP      # 4
NF = 2 * L                 # 16384
LOG_R_MIN, LOG_R_MAX = 0.0, 2.0

BF16 = ml_dtypes.bfloat16


def _make_consts():
    n2 = np.arange(64)
    n1 = np.arange(128)
    k1 = np.arange(128)
    k2 = np.arange(128)
    m64 = np.arange(64)

    Wc = np.exp(-2j * np.pi * np.outer(n2, k2) / 128)        # [64,128]
    T = np.exp(-2j * np.pi * np.outer(n1, k2) / NF)          # [128,128]
    W2 = np.exp(-2j * np.pi * np.outer(n1, k1) / 128)        # [128,128]
    Wcc = np.exp(+2j * np.pi * np.outer(k1, n1) / 128)       # [128,128]
    T2t = np.exp(+2j * np.pi * np.outer(k2, n1) / NF)        # [128,128]
    W2c = np.exp(+2j * np.pi * np.outer(k2, m64) / 128) / NF  # [128,64]

    bf = lambda a: np.ascontiguousarray(a, dtype=np.float32).astype(BF16)

    wblkA = np.zeros((128, 512))
    wblkB = np.zeros((128, 512))
    wblkA[0:64, 0:128] = Wc.real
    wblkA[0:64, 256:384] = Wc.imag
    wblkA[64:128, 128:256] = Wc.real
    wblkA[64:128, 384:512] = Wc.imag
    wblkB[0:64, 0:128] = -Wc.imag
    wblkB[0:64, 256:384] = Wc.real
    wblkB[64:128, 128:256] = -Wc.imag
    wblkB[64:128, 384:512] = Wc.real

    t_r2 = np.tile(T.real, (1, 2))
    t_i2 = np.tile(T.imag, (1, 2))
    t2_r2 = np.tile(T2t.real, (1, 2))
    t2_i2 = np.tile(T2t.imag, (1, 2))

    c = {}
    c["wblkA"] = bf(wblkA)
    c["wblkB"] = bf(wblkB)
    t_cat_a = np.concatenate([t_r2, t_i2], axis=1)             # [128,512]
    t_cat_b = np.concatenate([t_i2, t_r2], axis=1)
    t2_cat_a = np.concatenate([t2_r2, t2_i2], axis=1)
    t2_cat_b = np.concatenate([t2_i2, t2_r2], axis=1)
    c["t_cat_a2"] = bf(np.tile(t_cat_a, (1, 2)))               # [128,1024]
    c["t_cat_b2"] = bf(np.tile(t_cat_b, (1, 2)))
    c["t2_cat_a2"] = bf(np.tile(t2_cat_a, (1, 2)))
    c["t2_cat_b2"] = bf(np.tile(t2_cat_b, (1, 2)))
    c["w2_r"] = bf(W2.real)
    c["w2_i"] = bf(W2.imag)
    c["w2_ni"] = bf(-W2.imag)
    c["wcc_ri"] = bf(np.concatenate([Wcc.real, Wcc.imag], axis=1))    # [128,256]
    c["wcc_nir"] = bf(np.concatenate([-Wcc.imag, Wcc.real], axis=1))
    c["w2c_r"] = bf(W2c.real)       # [128,64]
    c["w2c_i"] = bf(W2c.imag)
    c["w2c_ni"] = bf(-W2c.imag)

    r = np.logspace(LOG_R_MIN, LOG_R_MAX, D).astype(np.float64)
    t = np.linspace(0.0, 1.0, L)
    decay = np.exp(-np.outer(r, t))
    c["_decay_full"] = np.ascontiguousarray(decay.astype(np.float32))
    return c


_CONSTS = _make_consts()
_NC_CACHE = {}

CONST_NAMES = ["wblkA", "wblkB", "t_cat_a2", "t_cat_b2", "t2_cat_a2",
               "t2_cat_b2", "w2_r", "w2_i", "w2_ni", "wcc_ri", "wcc_nir",
               "w2c_r", "w2c_i", "w2c_ni"]


def _build_nc():
    import concourse.bacc as bacc
    import concourse.tile as tile
    from concourse import mybir

    dt = mybir.dt
    AF = mybir.AluOpType

    nc = bacc.Bacc("TRN2", target_bir_lowering=False, debug=False,
                   num_devices=NCORES)

    def din(name, shape, d):
        return nc.dram_tensor(name, shape, d, kind="ExternalInput").ap()

    x1d = din("x1s", [B, DPC, L], dt.float32)
    x2d = din("x2s", [B, DPC, L], dt.float32)
    vd = din("vs", [B, DPC, L], dt.float32)
    hd = din("hs", [DPC, L], dt.float32)
    dbd = din("db_pair", [128, HALF], dt.float32)
    decd = din("decays", [DPC, L], dt.float32)
    cc = {}
    for nm in CONST_NAMES:
        shp = list(_CONSTS[nm].shape)
        cc[nm] = din(nm, shp, dt.bfloat16)
    outd = nc.dram_tensor("out", [B, DPC, L], dt.float32,
                          kind="ExternalOutput").ap()

    SW = SLABP * 256           # slab width for x-tensors (3072)
    KW = SLABP * 128           # slab width for h/decay (1536)

    def slab_in3h(eng, t, dram, s, h, b, jh):
        # column-half variant: pairs [jh*SLABP/2, (jh+1)*SLABP/2)
        jn = SLABP // 2
        dst = t[h * 64:(h + 1) * 64, jh * jn * 256:(jh + 1) * jn * 256].rearrange(
            "p (j b q) -> p j b q", j=jn, b=2, q=128)[:, :, b, :]
        c0 = s * SLABP + h * HALF + jh * jn
        src = dram[b, c0:c0 + jn, :]
        src = src.rearrange("j (p q) -> j p q", p=64, q=128).transpose([1, 0, 2])
        eng.dma_start(dst, src)

    def slab_in3(eng, t, dram, s, h, b):
        # t [128, SW]: partition (h:64)+p, col = j*256 + b*128 + q
        dst = t[h * 64:(h + 1) * 64, :].rearrange(
            "p (j b q) -> p j b q", j=SLABP, b=2, q=128)[:, :, b, :]
        src = dram[b, s * SLABP + h * HALF: s * SLABP + h * HALF + SLABP, :]
        src = src.rearrange("j (p q) -> j p q", p=64, q=128).transpose([1, 0, 2])
        eng.dma_start(dst, src)

    def slab_out3(eng, t, dram, s, h, b):
        dst = dram[b, s * SLABP + h * HALF: s * SLABP + h * HALF + SLABP, :]
        dst = dst.rearrange("j (p q) -> j p q", p=64, q=128).transpose([1, 0, 2])
        src = t[h * 64:(h + 1) * 64, :].rearrange(
            "p (j b q) -> p j b q", j=SLABP, b=2, q=128)[:, :, b, :]
        eng.dma_start(dst, src)

    def slab_in2(eng, t, dram, s, h):
        # t [128, KW]: partition (h:64)+p, col = j*128 + q
        dst = t[h * 64:(h + 1) * 64, :].rearrange(
            "p (j q) -> p j q", j=SLABP, q=128)
        src = dram[s * SLABP + h * HALF: s * SLABP + h * HALF + SLABP, :]
        src = src.rearrange("j (p q) -> j p q", p=64, q=128).transpose([1, 0, 2])
        eng.dma_start(dst, src)

    with tile.TileContext(nc, trace_sim=False) as tc:
        cpool = tc.alloc_tile_pool(name="consts", bufs=1)
        inpool = tc.alloc_tile_pool(name="inp", bufs=3)
        slabpool = tc.alloc_tile_pool(name="slab", bufs=2)
        fft = tc.alloc_tile_pool(name="fft", bufs=3)
        post = tc.alloc_tile_pool(name="post", bufs=3)
        ps2 = tc.alloc_tile_pool(name="ps2", bufs=2, space="PSUM")
        ps1 = tc.alloc_tile_pool(name="ps1", bufs=1, space="PSUM")

        csb = {}
        for nm, ap in cc.items():
            t = cpool.tile(list(ap.shape), dt.bfloat16, tag=nm)
            nc.sync.dma_start(t[:], ap)
            csb[nm] = t
        dbt = cpool.tile([128, HALF], dt.float32, tag="dbt")
        nc.sync.dma_start(dbt[:], dbd)

        for s in range(NSLAB):
            x2t = slabpool.tile([128, SW], dt.float32, tag="x2")
            vt = slabpool.tile([128, SW], dt.float32, tag="v")
            x1t = slabpool.tile([128, SW], dt.bfloat16, tag="x1")
            ht = slabpool.tile([128, KW], dt.bfloat16, tag="h")
            dct = slabpool.tile([128, KW], dt.bfloat16, tag="dec")
            for h in range(2):
                for b in range(2):
                    for jh in range(2):
                        slab_in3h(nc.sync, x2t, x2d, s, h, b, jh)
                        slab_in3h(nc.sync, vt, vd, s, h, b, jh)
                    slab_in3(nc.gpsimd, x1t, x1d, s, h, b)
                slab_in2(nc.gpsimd, ht, hd, s, h)
                slab_in2(nc.gpsimd, dct, decd, s, h)

            ut = slabpool.tile([128, SW], dt.bfloat16, tag="u")
            nc.vector.tensor_tensor(ut[:, 0:SW // 2], x2t[:, 0:SW // 2],
                                    vt[:, 0:SW // 2], AF.mult)
            nc.vector.tensor_tensor(ut[:, SW // 2:SW], x2t[:, SW // 2:SW],
                                    vt[:, SW // 2:SW], AF.mult)
            kt = slabpool.tile([128, KW], dt.bfloat16, tag="k")
            nc.vector.tensor_tensor(kt[:], ht[:], dct[:], AF.mult)

            outt = slabpool.tile([128, SW], dt.float32, tag="out")

            for d in range(SLABP // 2):
                # process a pair-double: pairs (2d, 2d+1); elementwise ops run
                # double-wide [128,1024]; matmuls/evacs stay per-pair (PSUM).
                js = (2 * d, 2 * d + 1)
                jc0 = 2 * d * 256

                udb = post.tile([128, 512], dt.bfloat16, tag="udb")
                z0b = fft.tile([128, 1024], dt.bfloat16, tag="z0b")
                k0b = fft.tile([128, 1024], dt.bfloat16, tag="k0b")
                for i, j in enumerate(js):
                    c = s * SLABP + j
                    jc = j * 256
                    nc.vector.tensor_scalar(udb[:, i * 256:(i + 1) * 256],
                                            ut[:, jc:jc + 256],
                                            dbt[:, c:c + 1], None, AF.mult)
                    # ---- S1 ----
                    z0 = ps1.tile([128, 512], dt.float32, tag="z0")
                    k0 = ps1.tile([128, 512], dt.float32, tag="k0")
                    nc.tensor.matmul(z0[:], ut[:, jc:jc + 128], csb["wblkA"][:],
                                     start=True, stop=False)
                    nc.tensor.matmul(z0[:], ut[:, jc + 128:jc + 256],
                                     csb["wblkB"][:], start=False, stop=True)
                    nc.tensor.matmul(k0[:], kt[:, j * 128:(j + 1) * 128],
                                     csb["wblkA"][:], start=True, stop=True)
                    nc.scalar.copy(z0b[:, i * 512:(i + 1) * 512], z0[:])
                    nc.scalar.copy(k0b[:, i * 512:(i + 1) * 512], k0[:])

                def iview(t, i):
                    # [128,1024] -> [128, 2, 256] selecting A(i=0)/B(i=1)
                    # halves of both pairs
                    return t[:].rearrange("p (d i q) -> p d i q",
                                          d=2, i=2, q=256)[:, :, i, :]

                # ---- forward twiddles (double-wide DVE) ----
                ma = fft.tile([128, 1024], dt.bfloat16, tag="ma")
                mb = fft.tile([128, 1024], dt.bfloat16, tag="mb")
                nc.vector.tensor_tensor(ma[:], z0b[:], csb["t_cat_a2"][:], AF.mult)
                nc.vector.tensor_tensor(mb[:], z0b[:], csb["t_cat_b2"][:], AF.mult)
                z1 = fft.tile([128, 1024], dt.bfloat16, tag="z1")
                nc.vector.tensor_tensor(iview(z1, 0), iview(ma, 0),
                                        iview(ma, 1), AF.subtract)
                nc.vector.tensor_tensor(iview(z1, 1), iview(mb, 0),
                                        iview(mb, 1), AF.add)
                kma = fft.tile([128, 1024], dt.bfloat16, tag="kma")
                kmb = fft.tile([128, 1024], dt.bfloat16, tag="kmb")
                nc.vector.tensor_tensor(kma[:], k0b[:], csb["t_cat_a2"][:], AF.mult)
                nc.vector.tensor_tensor(kmb[:], k0b[:], csb["t_cat_b2"][:], AF.mult)
                k1 = fft.tile([128, 1024], dt.bfloat16, tag="k1")
                nc.vector.tensor_tensor(iview(k1, 0), iview(kma, 0),
                                        iview(kma, 1), AF.subtract)
                nc.vector.tensor_tensor(iview(k1, 1), iview(kmb, 0),
                                        iview(kmb, 1), AF.add)

                # ---- S2 (per pair) + evacs into double tiles ----
                pzb = fft.tile([128, 1024], dt.bfloat16, tag="pzb")
                pkb = fft.tile([128, 1024], dt.bfloat16, tag="pkb")
                for i, j in enumerate(js):
                    z1s = z1[:, i * 512:(i + 1) * 512]
                    k1s = k1[:, i * 512:(i + 1) * 512]
                    pz = ps2.tile([128, 512], dt.float32, tag="pz")
                    pk = ps2.tile([128, 512], dt.float32, tag="pk")
                    nc.tensor.matmul(pz[:, 0:256], csb["w2_ni"][:],
                                     z1s[:, 256:512], start=True, stop=False)
                    nc.tensor.matmul(pz[:, 256:512], csb["w2_i"][:],
                                     z1s[:, 0:256], start=True, stop=False)
                    nc.tensor.matmul(pz[:], csb["w2_r"][:], z1s,
                                     start=False, stop=True)
                    nc.tensor.matmul(pk[:, 0:256], csb["w2_ni"][:],
                                     k1s[:, 256:512], start=True, stop=False)
                    nc.tensor.matmul(pk[:, 256:512], csb["w2_i"][:],
                                     k1s[:, 0:256], start=True, stop=False)
                    nc.tensor.matmul(pk[:], csb["w2_r"][:], k1s,
                                     start=False, stop=True)
                    nc.scalar.copy(pzb[:, i * 512:(i + 1) * 512], pz[:])
                    nc.scalar.copy(pkb[:, i * 512:(i + 1) * 512], pk[:])

                # ---- spectral product (double-wide, gpsimd mults) ----
                pa = fft.tile([128, 1024], dt.bfloat16, tag="pa")
                pb = fft.tile([128, 1024], dt.bfloat16, tag="pb")
                nc.gpsimd.tensor_tensor(pa[:], pzb[:], pkb[:], AF.mult)
                nc.gpsimd.tensor_tensor(iview(pb, 0), iview(pzb, 0),
                                        iview(pkb, 1), AF.mult)
                nc.gpsimd.tensor_tensor(iview(pb, 1), iview(pzb, 1),
                                        iview(pkb, 0), AF.mult)
                py = fft.tile([128, 1024], dt.bfloat16, tag="py")
                for i in range(2):
                    o = i * 512
                    nc.vector.tensor_tensor(py[:, o:o + 256], pa[:, o:o + 256],
                                            pa[:, o + 256:o + 512], AF.subtract)
                    nc.vector.tensor_tensor(py[:, o + 256:o + 512],
                                            pb[:, o:o + 256],
                                            pb[:, o + 256:o + 512], AF.add)

                # ---- S1' (per pair, strided PSUM out blocks) + evac ----
                atb = fft.tile([128, 1024], dt.bfloat16, tag="atb")
                for i, j in enumerate(js):
                    at = ps1.tile([128, 512], dt.float32, tag="at")
                    atv = at[:].rearrange("m (i c q) -> m i c q",
                                          i=2, c=2, q=128)
                    for ci in range(2):
                        blocks = atv[:, :, ci, :]
                        pyr = py[:, i * 512 + ci * 128:i * 512 + (ci + 1) * 128]
                        pyi = py[:, i * 512 + 256 + ci * 128:
                                 i * 512 + 256 + (ci + 1) * 128]
                        nc.tensor.matmul(blocks, pyi, csb["wcc_nir"][:],
                                         start=True, stop=False)
                        nc.tensor.matmul(blocks, pyr, csb["wcc_ri"][:],
                                         start=False, stop=True)
                    nc.scalar.copy(atb[:, i * 512:(i + 1) * 512], at[:])

                # ---- inverse twiddle (double-wide mults) ----
                ma2 = fft.tile([128, 1024], dt.bfloat16, tag="ma2")
                mb2 = fft.tile([128, 1024], dt.bfloat16, tag="mb2")
                nc.vector.tensor_tensor(ma2[:], atb[:], csb["t2_cat_a2"][:],
                                        AF.mult)
                nc.vector.tensor_tensor(mb2[:], atb[:], csb["t2_cat_b2"][:],
                                        AF.mult)
                bt = fft.tile([128, 1024], dt.bfloat16, tag="bt")
                for i in range(2):
                    o = i * 512
                    btv = bt[:, o:o + 512].rearrange(
                        "p (c i q) -> p c i q", c=2, i=2, q=128)
                    nc.vector.tensor_tensor(
                        btv[:, :, 0, :],
                        ma2[:, o:o + 256].rearrange("p (c q) -> p c q", c=2),
                        ma2[:, o + 256:o + 512].rearrange(
                            "p (c q) -> p c q", c=2), AF.subtract)
                    nc.vector.tensor_tensor(
                        btv[:, :, 1, :],
                        mb2[:, o:o + 256].rearrange("p (c q) -> p c q", c=2),
                        mb2[:, o + 256:o + 512].rearrange(
                            "p (c q) -> p c q", c=2), AF.add)

                # ---- S2' (per pair; high channel to PSUM rows 64:128) ----
                ygb = post.tile([128, 512], dt.bfloat16, tag="ygb")
                for i, j in enumerate(js):
                    yg = ps1.tile([128, 256], dt.float32, tag="yg")
                    for ci in range(2):
                        rows = yg[ci * 64:(ci + 1) * 64, :]
                        base = i * 512 + ci * 256
                        btr = bt[:, base:base + 128]
                        bti = bt[:, base + 128:base + 256]
                        nc.tensor.matmul(rows[:, 0:128], csb["w2c_ni"][:], bti,
                                         start=True, stop=False)
                        nc.tensor.matmul(rows[:, 128:256], csb["w2c_i"][:], btr,
                                         start=True, stop=False)
                        nc.tensor.matmul(rows[:], csb["w2c_r"][:],
                                         bt[:, base:base + 256],
                                         start=False, stop=True)
                    nc.scalar.copy(ygb[:, i * 256:(i + 1) * 256], yg[:])

                # ---- post: out = (y + db*u) * x1 (double-wide) ----
                tt = post.tile([128, 512], dt.bfloat16, tag="tt")
                nc.gpsimd.tensor_tensor(tt[:], udb[:], ygb[:], AF.add)
                nc.vector.tensor_tensor(outt[:, jc0:jc0 + 512], tt[:],
                                        x1t[:, jc0:jc0 + 512], AF.mult)

            for h in range(2):
                for b in range(2):
                    slab_out3(nc.scalar, outt, outd, s, h, b)

        for p in (ps1, ps2, post, fft, slabpool, inpool, cpool):
            p.release()

    nc.compile()
    return nc


def _get_nc():
    if "nc" not in _NC_CACHE:
        _NC_CACHE["nc"] = _build_nc()
    return _NC_CACHE["nc"]


def make_in_maps(x1, x2, v, h, d_bias):
    c = _CONSTS
    in_maps = []
    for core in range(NCORES):
        sl = slice(core * DPC, (core + 1) * DPC)
        db = d_bias[sl]
        db_pair = np.empty((128, HALF), np.float32)
        db_pair[0:64, :] = db[None, 0:HALF]
        db_pair[64:128, :] = db[None, HALF:DPC]
        m = {
            "x1s": np.ascontiguousarray(x1[:, sl]),
            "x2s": np.ascontiguousarray(x2[:, sl]),
            "vs": np.ascontiguousarray(v[:, sl]),
            "hs": np.ascontiguousarray(h[sl]),
            "db_pair": db_pair,
            "decays": np.ascontiguousarray(c["_decay_full"][sl]),
        }
        for nm in CONST_NAMES:
            m[nm] = c[nm]
        in_maps.append(m)
    return in_maps


def kernel(x1, x2, v, h, d_bias):
    from concourse import bass_utils

    x1 = np.ascontiguousarray(x1, dtype=np.float32)
    x2 = np.ascontiguousarray(x2, dtype=np.float32)
    v = np.ascontiguousarray(v, dtype=np.float32)
    h = np.ascontiguousarray(h, dtype=np.float32)
    d_bias = np.ascontiguousarray(d_bias, dtype=np.float32)

    nc = _get_nc()
    in_maps = make_in_maps(x1, x2, v, h, d_bias)
    res = bass_utils.run_bass_kernel_spmd(
        nc, in_maps, core_ids=list(range(NCORES)))
    out = np.concatenate([r["out"] for r in res.results], axis=1)
    return out.astype(np.float32)


if __name__ == "__main__":
    rng = np.random.default_rng(0)
    inputs = {
        "x1": rng.standard_normal((B, D, L)).astype(np.float32),
        "x2": rng.standard_normal((B, D, L)).astype(np.float32),
        "v": rng.standard_normal((B, D, L)).astype(np.float32),
        "h": (rng.standard_normal((D, L)) / math.sqrt(L) * 1e-5).astype(np.float32),
        "d_bias": rng.standard_normal(D).astype(np.float32),
    }
    out = kernel(**inputs)
    print(out.shape, out.dtype)


# revision 45
# speedup vs baseline: 1.0513x; 1.0513x over previous
"""ParallelHyenaOperator Trainium2 kernel.

out = (irfft(rfft(u,2L) * rfft(k,2L))[:L] + u*d_bias) * x1,  u = x2*v, k = h*decay

Sharding: D=768 channels split across 8 cores (96/core), no collectives.
Per core, channels are paired (c, c+48) and stacked in SBUF partitions
(c -> rows 0:64, c+48 -> rows 64:128), 8 pairs per slab, 6 slabs.
Each 16384-point FFT is a two-stage radix-128 factorization on the tensor
engine; both batches are packed as one complex series (z = u_b0 + i*u_b1).
Stage-1 matmuls take the stacked pair as the stationary operand against
block-diagonal DFT weights, producing both channels in one PSUM bank; the
final inverse stage writes the high channel to PSUM partitions 64:127
(PE tile_position col=64), so pre/post gating runs at full 128-partition
width. Twiddle/product stages run in bf16 on DVE (spectral-product
multiplies on GpSimd), double-wide over two pairs per op ([128,1024]) to
amortize per-op overhead; PSUM evacuations run on the scalar engine.
Inputs stream as whole slabs (4 dma_starts per tensor per slab); x1, h,
and decay are loaded as bf16 via gpsimd casting DMAs.

Measured on TRN2: ~359 us device exec (from 79.96 ms staged baseline);
rel err vs fp64 reference ~5.8e-3 (absmax-normalized), gate 2e-2.
"""

import math
import numpy as np
import ml_dtypes

B, D, L = 2, 768, 8192
NCORES = 8
DPC = D // NCORES          # 96 channels per core
HALF = DPC // 2            # 48; pairing (c, c+48)
SLABP = 8                  # pairs per slab
NSLAB = HALF // SLABP      # 4
NF = 2 * L                 # 16384
LOG_R_MIN, LOG_R_MAX = 0.0, 2.0

BF16 = ml_dtypes.bfloat16


def _make_consts():
    n2 = np.arange(64)
    n1 = np.arange(128)
    k1 = np.arange(128)
    k2 = np.arange(128)
    m64 = np.arange(64)

    Wc = np.exp(-2j * np.pi * np.outer(n2, k2) / 128)        # [64,128]
    T = np.exp(-2j * np.pi * np.outer(n1, k2) / NF)          # [128,128]
    W2 = np.exp(-2j * np.pi * np.outer(n1, k1) / 128)        # [128,128]
    Wcc = np.exp(+2j * np.pi * np.outer(k1, n1) / 128)       # [128,128]
    T2t = np.exp(+2j * np.pi * np.outer(k2, n1) / NF)        # [128,128]
    W2c = np.exp(+2j * np.pi * np.outer(k2, m64) / 128) / NF  # [128,64]

    bf = lambda a: np.ascontiguousarray(a, dtype=np.float32).astype(BF16)

    wblkA = np.zeros((128, 512))
    wblkB = np.zeros((128, 512))
    wblkA[0:64, 0:128] = Wc.real
    wblkA[0:64, 256:384] = Wc.imag
    wblkA[64:128, 128:256] = Wc.real
    wblkA[64:128, 384:512] = Wc.imag
    wblkB[0:64, 0:128] = -Wc.imag
    wblkB[0:64, 256:384] = Wc.real
    wblkB[64:128, 128:256] = -Wc.imag
    wblkB[64:128, 384:512] = Wc.real

    t_r2 = np.tile(T.real, (1, 2))
    t_i2 = np.tile(T.imag, (1, 2))
    t2_r2 = np.tile(T2t.real, (1, 2))
    t2_i2 = np.tile(T2t.imag, (1, 2))

    c = {}
    c["wblkA"] = bf(wblkA)
    c["wblkB"] = bf(wblkB)
    t_cat_a = np.concatenate([t_r2, t_i2], axis=1)             # [128,512]
    t_cat_b = np.concatenate([t_i2, t_r2], axis=1)
    t2_cat_a = np.concatenate([t2_r2, t2_i2], axis=1)
    t2_cat_b = np.concatenate([t2_i2, t2_r2], axis=1)
    c["t_cat_a2"] = bf(np.tile(t_cat_a, (1, 2)))               # [128,1024]
    c["t_cat_b2"] = bf(np.tile(t_cat_b, (1, 2)))
    c["t2_cat_a2"] = bf(np.tile(t2_cat_a, (1, 2)))
    c["t2_cat_b2"] = bf(np.tile(t2_cat_b, (1, 2)))
    c["w2_r"] = bf(W2.real)
    c["w2_i"] = bf(W2.imag)
    c["w2_ni"] = bf(-W2.imag)
    c["wcc_ri"] = bf(np.concatenate([Wcc.real, Wcc.imag], axis=1))    # [128,256]
    c["wcc_nir"] = bf(np.concatenate([-Wcc.imag, Wcc.real], axis=1))
    c["w2c_r"] = bf(W2c.real)       # [128,64]
    c["w2c_i"] = bf(W2c.imag)
    c["w2c_ni"] = bf(-W2c.imag)

    r = np.logspace(LOG_R_MIN, LOG_R_MAX, D).astype(np.float64)
    t = np.linspace(0.0, 1.0, L)
    decay = np.exp(-np.outer(r, t))
    c["_decay_full"] = np.ascontiguousarray(decay.astype(np.float32))
    return c


_CONSTS = _make_consts()
_NC_CACHE = {}

CONST_NAMES = ["wblkA", "wblkB", "t_cat_a2", "t_cat_b2", "t2_cat_a2",
               "t2_cat_b2", "w2_r", "w2_i", "w2_ni", "wcc_ri", "wcc_nir",
               "w2c_r", "w2c_i", "w2c_ni"]


def _build_nc():
    import concourse.bacc as bacc
    import concourse.tile as tile
    from concourse import mybir

    dt = mybir.dt
    AF = mybir.AluOpType

    nc = bacc.Bacc("TRN2", target_bir_lowering=False, debug=False,
                   num_devices=NCORES)

    def din(name, shape, d):
        return nc.dram_tensor(name, shape, d, kind="ExternalInput").ap()

    x1d = din("x1s", [B, DPC, L], dt.float32)
    x2d = din("x2s", [B, DPC, L], dt.float32)
    vd = din("vs", [B, DPC, L], dt.float32)
    hd = din("hs", [DPC, L], dt.float32)
    dbd = din("db_pair", [128, HALF], dt.float32)
    decd = din("decays", [DPC, L], dt.float32)
    cc = {}
    for nm in CONST_NAMES:
        shp = list(_CONSTS[nm].shape)
        cc[nm] = din(nm, shp, dt.bfloat16)
    outd = nc.dram_tensor("out", [B, DPC, L], dt.float32,
                          kind="ExternalOutput").ap()

    SW = SLABP * 256           # slab width for x-tensors (3072)
    KW = SLABP * 128           # slab width for h/decay (1536)

    def slab_in3(eng, t, dram, s, h, b):
        # t [128, SW]: partition (h:64)+p, col = j*256 + b*128 + q
        dst = t[h * 64:(h + 1) * 64, :].rearrange(
            "p (j b q) -> p j b q", j=SLABP, b=2, q=128)[:, :, b, :]
        src = dram[b, s * SLABP + h * HALF: s * SLABP + h * HALF + SLABP, :]
        src = src.rearrange("j (p q) -> j p q", p=64, q=128).transpose([1, 0, 2])
        eng.dma_start(dst, src)

    def slab_out3(eng, t, dram, s, h, b):
        dst = dram[b, s * SLABP + h * HALF: s * SLABP + h * HALF + SLABP, :]
        dst = dst.rearrange("j (p q) -> j p q", p=64, q=128).transpose([1, 0, 2])
        src = t[h * 64:(h + 1) * 64, :].rearrange(
            "p (j b q) -> p j b q", j=SLABP, b=2, q=128)[:, :, b, :]
        eng.dma_start(dst, src)

    def slab_in2(eng, t, dram, s, h):
        # t [128, KW]: partition (h:64)+p, col = j*128 + q
        dst = t[h * 64:(h + 1) * 64, :].rearrange(
            "p (j q) -> p j q", j=SLABP, q=128)
        src = dram[s * SLABP + h * HALF: s * SLABP + h * HALF + SLABP, :]
        src = src.rearrange("j (p q) -> j p q", p=64, q=128).transpose([1, 0, 2])
        eng.dma_start(dst, src)

    with tile.TileContext(nc, trace_sim=False) as tc:
        cpool = tc.alloc_tile_pool(name="consts", bufs=1)
        inpool = tc.alloc_tile_pool(name="inp", bufs=3)
        slabpool = tc.alloc_tile_pool(name="slab", bufs=2)
        fft = tc.alloc_tile_pool(name="fft", bufs=3)
        post = tc.alloc_tile_pool(name="post", bufs=3)
        ps2 = tc.alloc_tile_pool(name="ps2", bufs=2, space="PSUM")
        ps1 = tc.alloc_tile_pool(name="ps1", bufs=1, space="PSUM")

        csb = {}
        for nm, ap in cc.items():
            t = cpool.tile(list(ap.shape), dt.bfloat16, tag=nm)
            nc.sync.dma_start(t[:], ap)
            csb[nm] = t
        dbt = cpool.tile([128, HALF], dt.float32, tag="dbt")
        nc.sync.dma_start(dbt[:], dbd)

        for s in range(NSLAB):
            x2t = slabpool.tile([128, SW], dt.float32, tag="x2")
            vt = slabpool.tile([128, SW], dt.float32, tag="v")
            x1t = slabpool.tile([128, SW], dt.bfloat16, tag="x1")
            ht = slabpool.tile([128, KW], dt.bfloat16, tag="h")
            dct = slabpool.tile([128, KW], dt.bfloat16, tag="dec")
            for h in range(2):
                for b in range(2):
                    slab_in3(nc.sync, x2t, x2d, s, h, b)
                    slab_in3(nc.sync, vt, vd, s, h, b)
                    slab_in3(nc.gpsimd, x1t, x1d, s, h, b)
                slab_in2(nc.gpsimd, ht, hd, s, h)
                slab_in2(nc.gpsimd, dct, decd, s, h)

            ut = slabpool.tile([128, SW], dt.bfloat16, tag="u")
            nc.vector.tensor_tensor(ut[:], x2t[:], vt[:], AF.mult)
            kt = slabpool.tile([128, KW], dt.bfloat16, tag="k")
            nc.vector.tensor_tensor(kt[:], ht[:], dct[:], AF.mult)

            outt = slabpool.tile([128, SW], dt.float32, tag="out")

            for d in range(SLABP // 2):
                # process a pair-double: pairs (2d, 2d+1); elementwise ops run
                # double-wide [128,1024]; matmuls/evacs stay per-pair (PSUM).
                js = (2 * d, 2 * d + 1)
                jc0 = 2 * d * 256

                udb = post.tile([128, 512], dt.bfloat16, tag="udb")
                z0b = fft.tile([128, 1024], dt.bfloat16, tag="z0b")
                k0b = fft.tile([128, 1024], dt.bfloat16, tag="k0b")
                for i, j in enumerate(js):
                    c = s * SLABP + j
                    jc = j * 256
                    nc.vector.tensor_scalar(udb[:, i * 256:(i + 1) * 256],
                                            ut[:, jc:jc + 256],
                                            dbt[:, c:c + 1], None, AF.mult)
                    # ---- S1 ----
                    z0 = ps1.tile([128, 512], dt.float32, tag="z0")
                    k0 = ps1.tile([128, 512], dt.float32, tag="k0")
                    nc.tensor.matmul(z0[:], ut[:, jc:jc + 128], csb["wblkA"][:],
                                     start=True, stop=False)
                    nc.tensor.matmul(z0[:], ut[:, jc + 128:jc + 256],
                                     csb["wblkB"][:], start=False, stop=True)
                    nc.tensor.matmul(k0[:], kt[:, j * 128:(j + 1) * 128],
                                     csb["wblkA"][:], start=True, stop=True)
                    nc.scalar.copy(z0b[:, i * 512:(i + 1) * 512], z0[:])
                    nc.scalar.copy(k0b[:, i * 512:(i + 1) * 512], k0[:])

                def iview(t, i):
                    # [128,1024] -> [128, 2, 256] selecting A(i=0)/B(i=1)
                    # halves of both pairs
                    return t[:].rearrange("p (d i q) -> p d i q",
                                          d=2, i=2, q=256)[:, :, i, :]

                # ---- forward twiddles (double-wide DVE) ----
                ma = fft.tile([128, 1024], dt.bfloat16, tag="ma")
                mb = fft.tile([128, 1024], dt.bfloat16, tag="mb")
                nc.vector.tensor_tensor(ma[:], z0b[:], csb["t_cat_a2"][:], AF.mult)
                nc.vector.tensor_tensor(mb[:], z0b[:], csb["t_cat_b2"][:], AF.mult)
                z1 = fft.tile([128, 1024], dt.bfloat16, tag="z1")
                nc.vector.tensor_tensor(iview(z1, 0), iview(ma, 0),
                                        iview(ma, 1), AF.subtract)
                nc.vector.tensor_tensor(iview(z1, 1), iview(mb, 0),
                                        iview(mb, 1), AF.add)
                kma = fft.tile([128, 1024], dt.bfloat16, tag="kma")
                kmb = fft.tile([128, 1024], dt.bfloat16, tag="kmb")
                nc.vector.tensor_tensor(kma[:], k0b[:], csb["t_cat_a2"][:], AF.mult)
                nc.vector.tensor_tensor(kmb[:], k0b[:], csb["t_cat_b2"][:], AF.mult)
                k1 = fft.tile([128, 1024], dt.bfloat16, tag="k1")
                nc.vector.tensor_tensor(iview(k1, 0), iview(kma, 0),
                                        iview(kma, 1), AF.subtract)
                nc.vector.tensor_tensor(iview(k1, 1), iview(kmb, 0),
                                        iview(kmb, 1), AF.add)

                # ---- S2 (per pair) + evacs into double tiles ----
                pzb = fft.tile([128, 1024], dt.bfloat16, tag="pzb")
                pkb = fft.tile([128, 1024], dt.bfloat16, tag="pkb")
                for i, j in enumerate(js):
                    z1s = z1[:, i * 512:(i + 1) * 512]
                    k1s = k1[:, i * 512:(i + 1) * 512]
                    pz = ps2.tile([128, 512], dt.float32, tag="pz")
                    pk = ps2.tile([128, 512], dt.float32, tag="pk")
                    nc.tensor.matmul(pz[:, 0:256], csb["w2_ni"][:],
                                     z1s[:, 256:512], start=True, stop=False)
                    nc.tensor.matmul(pz[:, 256:512], csb["w2_i"][:],
                                     z1s[:, 0:256], start=True, stop=False)
                    nc.tensor.matmul(pz[:], csb["w2_r"][:], z1s,
                                     start=False, stop=True)
                    nc.tensor.matmul(pk[:, 0:256], csb["w2_ni"][:],
                                     k1s[:, 256:512], start=True, stop=False)
                    nc.tensor.matmul(pk[:, 256:512], csb["w2_i"][:],
                                     k1s[:, 0:256], start=True, stop=False)
                    nc.tensor.matmul(pk[:], csb["w2_r"][:], k1s,
                                     start=False, stop=True)
                    nc.scalar.copy(pzb[:, i * 512:(i + 1) * 512], pz[:])
                    nc.scalar.copy(pkb[:, i * 512:(i + 1) * 512], pk[:])

                # ---- spectral product (double-wide, gpsimd mults) ----
                pa = fft.tile([128, 1024], dt.bfloat16, tag="pa")
                pb = fft.tile([128, 1024], dt.bfloat16, tag="pb")
                nc.gpsimd.tensor_tensor(pa[:], pzb[:], pkb[:], AF.mult)
                nc.gpsimd.tensor_tensor(iview(pb, 0), iview(pzb, 0),
                                        iview(pkb, 1), AF.mult)
                nc.gpsimd.tensor_tensor(iview(pb, 1), iview(pzb, 1),
                                        iview(pkb, 0), AF.mult)
                py = fft.tile([128, 1024], dt.bfloat16, tag="py")
                for i in range(2):
                    o = i * 512
                    nc.vector.tensor_tensor(py[:, o:o + 256], pa[:, o:o + 256],
                                            pa[:, o + 256:o + 512], AF.subtract)
                    nc.vector.tensor_tensor(py[:, o + 256:o + 512],
                                            pb[:, o:o + 256],
                                            pb[:, o + 256:o + 512], AF.add)

                # ---- S1' (per pair, strided PSUM out blocks) + evac ----
                atb = fft.tile([128, 1024], dt.bfloat16, tag="atb")
                for i, j in enumerate(js):
                    at = ps1.tile([128, 512], dt.float32, tag="at")
                    atv = at[:].rearrange("m (i c q) -> m i c q",
                                          i=2, c=2, q=128)
                    for ci in range(2):
                        blocks = atv[:, :, ci, :]
                        pyr = py[:, i * 512 + ci * 128:i * 512 + (ci + 1) * 128]
                        pyi = py[:, i * 512 + 256 + ci * 128:
                                 i * 512 + 256 + (ci + 1) * 128]
                        nc.tensor.matmul(blocks, pyi, csb["wcc_nir"][:],
                                         start=True, stop=False)
                        nc.tensor.matmul(blocks, pyr, csb["wcc_ri"][:],
                                         start=False, stop=True)
                    nc.scalar.copy(atb[:, i * 512:(i + 1) * 512], at[:])

                # ---- inverse twiddle (double-wide mults) ----
                ma2 = fft.tile([128, 1024], dt.bfloat16, tag="ma2")
                mb2 = fft.tile([128, 1024], dt.bfloat16, tag="mb2")
                nc.vector.tensor_tensor(ma2[:], atb[:], csb["t2_cat_a2"][:],
                                        AF.mult)
                nc.vector.tensor_tensor(mb2[:], atb[:], csb["t2_cat_b2"][:],
                                        AF.mult)
                bt = fft.tile([128, 1024], dt.bfloat16, tag="bt")
                for i in range(2):
                    o = i * 512
                    btv = bt[:, o:o + 512].rearrange(
                        "p (c i q) -> p c i q", c=2, i=2, q=128)
                    nc.vector.tensor_tensor(
                        btv[:, :, 0, :],
                        ma2[:, o:o + 256].rearrange("p (c q) -> p c q", c=2),
                        ma2[:, o + 256:o + 512].rearrange(
                            "p (c q) -> p c q", c=2), AF.subtract)
                    nc.vector.tensor_tensor(
                        btv[:, :, 1, :],
                        mb2[:, o:o + 256].rearrange("p (c q) -> p c q", c=2),
                        mb2[:, o + 256:o + 512].rearrange(
                            "p (c q) -> p c q", c=2), AF.add)

                # ---- S2' (per pair; high channel to PSUM rows 64:128) ----
                ygb = post.tile([128, 512], dt.bfloat16, tag="ygb")
                for i, j in enumerate(js):
                    yg = ps1.tile([128, 256], dt.float32, tag="yg")
                    for ci in range(2):
                        rows = yg[ci * 64:(ci + 1) * 64, :]
                        base = i * 512 + ci * 256
                        btr = bt[:, base:base + 128]
                        bti = bt[:, base + 128:base + 256]
                        nc.tensor.matmul(rows[:, 0:128], csb["w2c_ni"][:], bti,
                                         start=True, stop=False)
                        nc.tensor.matmul(rows[:, 128:256], csb["w2c_i"][:], btr,
                                         start=True, stop=False)
                        nc.tensor.matmul(rows[:], csb["w2c_r"][:],
                                         bt[:, base:base + 256],
                                         start=False, stop=True)
                    nc.scalar.copy(ygb[:, i * 256:(i + 1) * 256], yg[:])

                # ---- post: out = (y + db*u) * x1 (double-wide) ----
                tt = post.tile([128, 512], dt.bfloat16, tag="tt")
                nc.vector.tensor_tensor(tt[:], udb[:], ygb[:], AF.add)
                nc.vector.tensor_tensor(outt[:, jc0:jc0 + 512], tt[:],
                                        x1t[:, jc0:jc0 + 512], AF.mult)

            for h in range(2):
                for b in range(2):
                    slab_out3(nc.scalar, outt, outd, s, h, b)

        for p in (ps1, ps2, post, fft, slabpool, inpool, cpool):
            p.release()

    nc.compile()
    return nc


def _get_nc():
    if "nc" not in _NC_CACHE:
        _NC_CACHE["nc"] = _build_nc()
    return _NC_CACHE["nc"]


def make_in_maps(x1, x2, v, h, d_bias):
    c = _CONSTS
    in_maps = []
    for core in range(NCORES):
        sl = slice(core * DPC, (core + 1) * DPC)
        db = d_bias[sl]
        db_pair = np.empty((128, HALF), np.float32)
        db_pair[0:64, :] = db[None, 0:HALF]
        db_pair[64:128, :] = db[None, HALF:DPC]
        m = {
            "x1s": np.ascontiguousarray(x1[:, sl]),
            "x2s": np.ascontiguousarray(x2[:, sl]),
            "vs": np.ascontiguousarray(v[:, sl]),
            "hs": np.ascontiguousarray(h[sl]),
            "db_pair": db_pair,
            "decays": np.ascontiguousarray(c["_decay_full"][sl]),
        }
        for nm in CONST_NAMES:
            m[nm] = c[nm]
        in_maps.append(m)
    return in_maps


def kernel(x1, x2, v, h, d_bias):
    from concourse import bass_utils

    x1 = np.ascontiguousarray(x1, dtype=np.float32)
    x2 = np.ascontiguousarray(x2, dtype=np.float32)
    v = np.ascontiguousarray(v, dtype=np.float32)
    h = np.ascontiguousarray(h, dtype=np.float32)
    d_bias = np.ascontiguousarray(d_bias, dtype=np.float32)

    nc = _get_nc()
    in_maps = make_in_maps(x1, x2, v, h, d_bias)
    res = bass_utils.run_bass_kernel_spmd(
        nc, in_maps, core_ids=list(range(NCORES)))
    out = np.concatenate([r["out"] for r in res.results], axis=1)
    return out.astype(np.float32)


if __name__ == "__main__":
    rng = np.random.default_rng(0)
    inputs = {
        "x1": rng.standard_normal((B, D, L)).astype(np.float32),
        "x2": rng.standard_normal((B, D, L)).astype(np.float32),
        "v": rng.standard_normal((B, D, L)).astype(np.float32),
        "h": (rng.standard_normal((D, L)) / math.sqrt(L) * 1e-5).astype(np.float32),
        "d_bias": rng.standard_normal(D).astype(np.float32),
    }
    out = kernel(**inputs)
    print(out.shape, out.dtype)


# revision 46
# speedup vs baseline: 1.0598x; 1.0080x over previous
"""ParallelHyenaOperator Trainium2 kernel.

out = (irfft(rfft(u,2L) * rfft(k,2L))[:L] + u*d_bias) * x1,  u = x2*v, k = h*decay

Sharding: D=768 channels split across 8 cores (96/core), no collectives.
Per core, channels are paired (c, c+48) and stacked in SBUF partitions
(c -> rows 0:64, c+48 -> rows 64:128), 8 pairs per slab, 6 slabs.
Each 16384-point FFT is a two-stage radix-128 factorization on the tensor
engine; both batches are packed as one complex series (z = u_b0 + i*u_b1).
Stage-1 matmuls take the stacked pair as the stationary operand against
block-diagonal DFT weights, producing both channels in one PSUM bank; the
final inverse stage writes the high channel to PSUM partitions 64:127
(PE tile_position col=64), so pre/post gating runs at full 128-partition
width. Twiddle/product stages run in bf16 on DVE (spectral-product
multiplies on GpSimd), double-wide over two pairs per op ([128,1024]) to
amortize per-op overhead; PSUM evacuations run on the scalar engine.
Inputs stream as whole slabs (4 dma_starts per tensor per slab); x1, h,
and decay are loaded as bf16 via gpsimd casting DMAs.

Measured on TRN2: ~359 us device exec (from 79.96 ms staged baseline);
rel err vs fp64 reference ~5.8e-3 (absmax-normalized), gate 2e-2.
"""

import math
import numpy as np
import ml_dtypes

B, D, L = 2, 768, 8192
NCORES = 8
DPC = D // NCORES          # 96 channels per core
HALF = DPC // 2            # 48; pairing (c, c+48)
SLABP = 8                  # pairs per slab
NSLAB = HALF // SLABP      # 4
NF = 2 * L                 # 16384
LOG_R_MIN, LOG_R_MAX = 0.0, 2.0

BF16 = ml_dtypes.bfloat16


def _make_consts():
    n2 = np.arange(64)
    n1 = np.arange(128)
    k1 = np.arange(128)
    k2 = np.arange(128)
    m64 = np.arange(64)

    Wc = np.exp(-2j * np.pi * np.outer(n2, k2) / 128)        # [64,128]
    T = np.exp(-2j * np.pi * np.outer(n1, k2) / NF)          # [128,128]
    W2 = np.exp(-2j * np.pi * np.outer(n1, k1) / 128)        # [128,128]
    Wcc = np.exp(+2j * np.pi * np.outer(k1, n1) / 128)       # [128,128]
    T2t = np.exp(+2j * np.pi * np.outer(k2, n1) / NF)        # [128,128]
    W2c = np.exp(+2j * np.pi * np.outer(k2, m64) / 128) / NF  # [128,64]

    bf = lambda a: np.ascontiguousarray(a, dtype=np.float32).astype(BF16)

    wblkA = np.zeros((128, 512))
    wblkB = np.zeros((128, 512))
    wblkA[0:64, 0:128] = Wc.real
    wblkA[0:64, 256:384] = Wc.imag
    wblkA[64:128, 128:256] = Wc.real
    wblkA[64:128, 384:512] = Wc.imag
    wblkB[0:64, 0:128] = -Wc.imag
    wblkB[0:64, 256:384] = Wc.real
    wblkB[64:128, 128:256] = -Wc.imag
    wblkB[64:128, 384:512] = Wc.real

    t_r2 = np.tile(T.real, (1, 2))
    t_i2 = np.tile(T.imag, (1, 2))
    t2_r2 = np.tile(T2t.real, (1, 2))
    t2_i2 = np.tile(T2t.imag, (1, 2))

    c = {}
    c["wblkA"] = bf(wblkA)
    c["wblkB"] = bf(wblkB)
    t_cat_a = np.concatenate([t_r2, t_i2], axis=1)             # [128,512]
    t_cat_b = np.concatenate([t_i2, t_r2], axis=1)
    t2_cat_a = np.concatenate([t2_r2, t2_i2], axis=1)
    t2_cat_b = np.concatenate([t2_i2, t2_r2], axis=1)
    c["t_cat_a2"] = bf(np.tile(t_cat_a, (1, 2)))               # [128,1024]
    c["t_cat_b2"] = bf(np.tile(t_cat_b, (1, 2)))
    c["t2_cat_a2"] = bf(np.tile(t2_cat_a, (1, 2)))
    c["t2_cat_b2"] = bf(np.tile(t2_cat_b, (1, 2)))
    c["w2_r"] = bf(W2.real)
    c["w2_i"] = bf(W2.imag)
    c["w2_ni"] = bf(-W2.imag)
    c["wcc_ri"] = bf(np.concatenate([Wcc.real, Wcc.imag], axis=1))    # [128,256]
    c["wcc_nir"] = bf(np.concatenate([-Wcc.imag, Wcc.real], axis=1))
    c["w2c_r"] = bf(W2c.real)       # [128,64]
    c["w2c_i"] = bf(W2c.imag)
    c["w2c_ni"] = bf(-W2c.imag)

    r = np.logspace(LOG_R_MIN, LOG_R_MAX, D).astype(np.float64)
    t = np.linspace(0.0, 1.0, L)
    decay = np.exp(-np.outer(r, t))
    c["_decay_full"] = np.ascontiguousarray(decay.astype(np.float32))
    return c


_CONSTS = _make_consts()
_NC_CACHE = {}

CONST_NAMES = ["wblkA", "wblkB", "t_cat_a2", "t_cat_b2", "t2_cat_a2",
               "t2_cat_b2", "w2_r", "w2_i", "w2_ni", "wcc_ri", "wcc_nir",
               "w2c_r", "w2c_i", "w2c_ni"]


def _build_nc():
    import concourse.bacc as bacc
    import concourse.tile as tile
    from concourse import mybir

    dt = mybir.dt
    AF = mybir.AluOpType

    nc = bacc.Bacc("TRN2", target_bir_lowering=False, debug=False,
                   num_devices=NCORES)

    def din(name, shape, d):
        return nc.dram_tensor(name, shape, d, kind="ExternalInput").ap()

    x1d = din("x1s", [B, DPC, L], dt.float32)
    x2d = din("x2s", [B, DPC, L], dt.float32)
    vd = din("vs", [B, DPC, L], dt.float32)
    hd = din("hs", [DPC, L], dt.float32)
    dbd = din("db_pair", [128, HALF], dt.float32)
    decd = din("decays", [DPC, L], dt.float32)
    cc = {}
    for nm in CONST_NAMES:
        shp = list(_CONSTS[nm].shape)
        cc[nm] = din(nm, shp, dt.bfloat16)
    outd = nc.dram_tensor("out", [B, DPC, L], dt.float32,
                          kind="ExternalOutput").ap()

    SW = SLABP * 256           # slab width for x-tensors (3072)
    KW = SLABP * 128           # slab width for h/decay (1536)

    def slab_in3(eng, t, dram, s, h, b):
        # t [128, SW]: partition (h:64)+p, col = j*256 + b*128 + q
        dst = t[h * 64:(h + 1) * 64, :].rearrange(
            "p (j b q) -> p j b q", j=SLABP, b=2, q=128)[:, :, b, :]
        src = dram[b, s * SLABP + h * HALF: s * SLABP + h * HALF + SLABP, :]
        src = src.rearrange("j (p q) -> j p q", p=64, q=128).transpose([1, 0, 2])
        eng.dma_start(dst, src)

    def slab_out3(eng, t, dram, s, h, b):
        dst = dram[b, s * SLABP + h * HALF: s * SLABP + h * HALF + SLABP, :]
        dst = dst.rearrange("j (p q) -> j p q", p=64, q=128).transpose([1, 0, 2])
        src = t[h * 64:(h + 1) * 64, :].rearrange(
            "p (j b q) -> p j b q", j=SLABP, b=2, q=128)[:, :, b, :]
        eng.dma_start(dst, src)

    def slab_in2(eng, t, dram, s, h):
        # t [128, KW]: partition (h:64)+p, col = j*128 + q
        dst = t[h * 64:(h + 1) * 64, :].rearrange(
            "p (j q) -> p j q", j=SLABP, q=128)
        src = dram[s * SLABP + h * HALF: s * SLABP + h * HALF + SLABP, :]
        src = src.rearrange("j (p q) -> j p q", p=64, q=128).transpose([1, 0, 2])
        eng.dma_start(dst, src)

    with tile.TileContext(nc, trace_sim=False) as tc:
        cpool = tc.alloc_tile_pool(name="consts", bufs=1)
        inpool = tc.alloc_tile_pool(name="inp", bufs=3)
        slabpool = tc.alloc_tile_pool(name="slab", bufs=2)
        fft = tc.alloc_tile_pool(name="fft", bufs=3)
        post = tc.alloc_tile_pool(name="post", bufs=3)
        ps2 = tc.alloc_tile_pool(name="ps2", bufs=2, space="PSUM")
        ps1 = tc.alloc_tile_pool(name="ps1", bufs=1, space="PSUM")

        csb = {}
        for nm, ap in cc.items():
            t = cpool.tile(list(ap.shape), dt.bfloat16, tag=nm)
            nc.sync.dma_start(t[:], ap)
            csb[nm] = t
        dbt = cpool.tile([128, HALF], dt.float32, tag="dbt")
        nc.sync.dma_start(dbt[:], dbd)

        for s in range(NSLAB):
            x2t = slabpool.tile([128, SW], dt.float32, tag="x2")
            vt = slabpool.tile([128, SW], dt.float32, tag="v")
            x1t = slabpool.tile([128, SW], dt.bfloat16, tag="x1")
            ht = slabpool.tile([128, KW], dt.bfloat16, tag="h")
            dct = slabpool.tile([128, KW], dt.bfloat16, tag="dec")
            for h in range(2):
                for b in range(2):
                    slab_in3(nc.sync, x2t, x2d, s, h, b)
                    slab_in3(nc.sync, vt, vd, s, h, b)
                    slab_in3(nc.gpsimd, x1t, x1d, s, h, b)
                slab_in2(nc.gpsimd, ht, hd, s, h)
                slab_in2(nc.gpsimd, dct, decd, s, h)

            ut = slabpool.tile([128, SW], dt.bfloat16, tag="u")
            nc.vector.tensor_tensor(ut[:], x2t[:], vt[:], AF.mult)
            kt = slabpool.tile([128, KW], dt.bfloat16, tag="k")
            nc.vector.tensor_tensor(kt[:], ht[:], dct[:], AF.mult)

            outt = slabpool.tile([128, SW], dt.float32, tag="out")

            for d in range(SLABP // 2):
                # process a pair-double: pairs (2d, 2d+1); elementwise ops run
                # double-wide [128,1024]; matmuls/evacs stay per-pair (PSUM).
                js = (2 * d, 2 * d + 1)
                jc0 = 2 * d * 256

                udb = post.tile([128, 512], dt.bfloat16, tag="udb")
                z0b = fft.tile([128, 1024], dt.bfloat16, tag="z0b")
                k0b = fft.tile([128, 1024], dt.bfloat16, tag="k0b")
                for i, j in enumerate(js):
                    c = s * SLABP + j
                    jc = j * 256
                    nc.vector.tensor_scalar(udb[:, i * 256:(i + 1) * 256],
                                            ut[:, jc:jc + 256],
                                            dbt[:, c:c + 1], None, AF.mult)
                    # ---- S1 ----
                    z0 = ps1.tile([128, 512], dt.float32, tag="z0")
                    k0 = ps1.tile([128, 512], dt.float32, tag="k0")
                    nc.tensor.matmul(z0[:], ut[:, jc:jc + 128], csb["wblkA"][:],
                                     start=True, stop=False)
                    nc.tensor.matmul(z0[:], ut[:, jc + 128:jc + 256],
                                     csb["wblkB"][:], start=False, stop=True)
                    nc.tensor.matmul(k0[:], kt[:, j * 128:(j + 1) * 128],
                                     csb["wblkA"][:], start=True, stop=True)
                    nc.scalar.copy(z0b[:, i * 512:(i + 1) * 512], z0[:])
                    nc.scalar.copy(k0b[:, i * 512:(i + 1) * 512], k0[:])

                def iview(t, i):
                    # [128,1024] -> [128, 2, 256] selecting A(i=0)/B(i=1)
                    # halves of both pairs
                    return t[:].rearrange("p (d i q) -> p d i q",
                                          d=2, i=2, q=256)[:, :, i, :]

                # ---- forward twiddles (double-wide DVE) ----
                ma = fft.tile([128, 1024], dt.bfloat16, tag="ma")
                mb = fft.tile([128, 1024], dt.bfloat16, tag="mb")
                nc.vector.tensor_tensor(ma[:], z0b[:], csb["t_cat_a2"][:], AF.mult)
                nc.vector.tensor_tensor(mb[:], z0b[:], csb["t_cat_b2"][:], AF.mult)
                z1 = fft.tile([128, 1024], dt.bfloat16, tag="z1")
                nc.vector.tensor_tensor(iview(z1, 0), iview(ma, 0),
                                        iview(ma, 1), AF.subtract)
                nc.vector.tensor_tensor(iview(z1, 1), iview(mb, 0),
                                        iview(mb, 1), AF.add)
                kma = fft.tile([128, 1024], dt.bfloat16, tag="kma")
                kmb = fft.tile([128, 1024], dt.bfloat16, tag="kmb")
                nc.vector.tensor_tensor(kma[:], k0b[:], csb["t_cat_a2"][:], AF.mult)
                nc.vector.tensor_tensor(kmb[:], k0b[:], csb["t_cat_b2"][:], AF.mult)
                k1 = fft.tile([128, 1024], dt.bfloat16, tag="k1")
                nc.vector.tensor_tensor(iview(k1, 0), iview(kma, 0),
                                        iview(kma, 1), AF.subtract)
                nc.vector.tensor_tensor(iview(k1, 1), iview(kmb, 0),
                                        iview(kmb, 1), AF.add)

                # ---- S2 (per pair) + evacs into double tiles ----
                pzb = fft.tile([128, 1024], dt.bfloat16, tag="pzb")
                pkb = fft.tile([128, 1024], dt.bfloat16, tag="pkb")
                for i, j in enumerate(js):
                    z1s = z1[:, i * 512:(i + 1) * 512]
                    k1s = k1[:, i * 512:(i + 1) * 512]
                    pz = ps2.tile([128, 512], dt.float32, tag="pz")
                    pk = ps2.tile([128, 512], dt.float32, tag="pk")
                    nc.tensor.matmul(pz[:, 0:256], csb["w2_ni"][:],
                                     z1s[:, 256:512], start=True, stop=False)
                    nc.tensor.matmul(pz[:, 256:512], csb["w2_i"][:],
                                     z1s[:, 0:256], start=True, stop=False)
                    nc.tensor.matmul(pz[:], csb["w2_r"][:], z1s,
                                     start=False, stop=True)
                    nc.tensor.matmul(pk[:, 0:256], csb["w2_ni"][:],
                                     k1s[:, 256:512], start=True, stop=False)
                    nc.tensor.matmul(pk[:, 256:512], csb["w2_i"][:],
                                     k1s[:, 0:256], start=True, stop=False)
                    nc.tensor.matmul(pk[:], csb["w2_r"][:], k1s,
                                     start=False, stop=True)
                    nc.scalar.copy(pzb[:, i * 512:(i + 1) * 512], pz[:])
                    nc.scalar.copy(pkb[:, i * 512:(i + 1) * 512], pk[:])

                # ---- spectral product (double-wide, gpsimd mults) ----
                pa = fft.tile([128, 1024], dt.bfloat16, tag="pa")
                pb = fft.tile([128, 1024], dt.bfloat16, tag="pb")
                nc.gpsimd.tensor_tensor(pa[:], pzb[:], pkb[:], AF.mult)
                nc.gpsimd.tensor_tensor(iview(pb, 0), iview(pzb, 0),
                                        iview(pkb, 1), AF.mult)
                nc.gpsimd.tensor_tensor(iview(pb, 1), iview(pzb, 1),
                                        iview(pkb, 0), AF.mult)
                py = fft.tile([128, 1024], dt.bfloat16, tag="py")
                nc.vector.tensor_tensor(iview(py, 0), iview(pa, 0),
                                        iview(pa, 1), AF.subtract)
                nc.vector.tensor_tensor(iview(py, 1), iview(pb, 0),
                                        iview(pb, 1), AF.add)

                # ---- S1' (per pair, strided PSUM out blocks) + evac ----
                atb = fft.tile([128, 1024], dt.bfloat16, tag="atb")
                for i, j in enumerate(js):
                    at = ps1.tile([128, 512], dt.float32, tag="at")
                    atv = at[:].rearrange("m (i c q) -> m i c q",
                                          i=2, c=2, q=128)
                    for ci in range(2):
                        blocks = atv[:, :, ci, :]
                        pyr = py[:, i * 512 + ci * 128:i * 512 + (ci + 1) * 128]
                        pyi = py[:, i * 512 + 256 + ci * 128:
                                 i * 512 + 256 + (ci + 1) * 128]
                        nc.tensor.matmul(blocks, pyi, csb["wcc_nir"][:],
                                         start=True, stop=False)
                        nc.tensor.matmul(blocks, pyr, csb["wcc_ri"][:],
                                         start=False, stop=True)
                    nc.scalar.copy(atb[:, i * 512:(i + 1) * 512], at[:])

                # ---- inverse twiddle (double-wide mults) ----
                ma2 = fft.tile([128, 1024], dt.bfloat16, tag="ma2")
                mb2 = fft.tile([128, 1024], dt.bfloat16, tag="mb2")
                nc.vector.tensor_tensor(ma2[:], atb[:], csb["t2_cat_a2"][:],
                                        AF.mult)
                nc.vector.tensor_tensor(mb2[:], atb[:], csb["t2_cat_b2"][:],
                                        AF.mult)
                # bt layout: per pair-half [btr_c | btr_ch | bti_c | bti_ch]
                bt = fft.tile([128, 1024], dt.bfloat16, tag="bt")
                nc.vector.tensor_tensor(iview(bt, 0), iview(ma2, 0),
                                        iview(ma2, 1), AF.subtract)
                nc.vector.tensor_tensor(iview(bt, 1), iview(mb2, 0),
                                        iview(mb2, 1), AF.add)

                # ---- S2' (per pair; high channel to PSUM rows 64:128) ----
                ygb = post.tile([128, 512], dt.bfloat16, tag="ygb")
                btq = bt[:].rearrange("p (i r c q) -> p i r c q",
                                      i=2, r=2, c=2, q=128)
                for i, j in enumerate(js):
                    yg = ps1.tile([128, 256], dt.float32, tag="yg")
                    for ci in range(2):
                        rows = yg[ci * 64:(ci + 1) * 64, :]
                        btr = bt[:, i * 512 + ci * 128:i * 512 + ci * 128 + 128]
                        bti = bt[:, i * 512 + 256 + ci * 128:
                                 i * 512 + 256 + ci * 128 + 128]
                        nc.tensor.matmul(rows[:, 0:128], csb["w2c_ni"][:], bti,
                                         start=True, stop=False)
                        nc.tensor.matmul(rows[:, 128:256], csb["w2c_i"][:], btr,
                                         start=True, stop=False)
                        nc.tensor.matmul(rows[:], csb["w2c_r"][:],
                                         btq[:, i, :, ci, :],
                                         start=False, stop=True)
                    nc.scalar.copy(ygb[:, i * 256:(i + 1) * 256], yg[:])

                # ---- post: out = (y + db*u) * x1 (double-wide) ----
                tt = post.tile([128, 512], dt.bfloat16, tag="tt")
                nc.vector.tensor_tensor(tt[:], udb[:], ygb[:], AF.add)
                nc.vector.tensor_tensor(outt[:, jc0:jc0 + 512], tt[:],
                                        x1t[:, jc0:jc0 + 512], AF.mult)

            for h in range(2):
                for b in range(2):
                    slab_out3(nc.scalar, outt, outd, s, h, b)

        for p in (ps1, ps2, post, fft, slabpool, inpool, cpool):
            p.release()

    nc.compile()
    return nc


def _get_nc():
    if "nc" not in _NC_CACHE:
        _NC_CACHE["nc"] = _build_nc()
    return _NC_CACHE["nc"]


def make_in_maps(x1, x2, v, h, d_bias):
    c = _CONSTS
    in_maps = []
    for core in range(NCORES):
        sl = slice(core * DPC, (core + 1) * DPC)
        db = d_bias[sl]
        db_pair = np.empty((128, HALF), np.float32)
        db_pair[0:64, :] = db[None, 0:HALF]
        db_pair[64:128, :] = db[None, HALF:DPC]
        m = {
            "x1s": np.ascontiguousarray(x1[:, sl]),
            "x2s": np.ascontiguousarray(x2[:, sl]),
            "vs": np.ascontiguousarray(v[:, sl]),
            "hs": np.ascontiguousarray(h[sl]),
            "db_pair": db_pair,
            "decays": np.ascontiguousarray(c["_decay_full"][sl]),
        }
        for nm in CONST_NAMES:
            m[nm] = c[nm]
        in_maps.append(m)
    return in_maps


def kernel(x1, x2, v, h, d_bias):
    from concourse import bass_utils

    x1 = np.ascontiguousarray(x1, dtype=np.float32)
    x2 = np.ascontiguousarray(x2, dtype=np.float32)
    v = np.ascontiguousarray(v, dtype=np.float32)
    h = np.ascontiguousarray(h, dtype=np.float32)
    d_bias = np.ascontiguousarray(d_bias, dtype=np.float32)

    nc = _get_nc()
    in_maps = make_in_maps(x1, x2, v, h, d_bias)
    res = bass_utils.run_bass_kernel_spmd(
        nc, in_maps, core_ids=list(range(NCORES)))
    out = np.concatenate([r["out"] for r in res.results], axis=1)
    return out.astype(np.float32)


if __name__ == "__main__":
    rng = np.random.default_rng(0)
    inputs = {
        "x1": rng.standard_normal((B, D, L)).astype(np.float32),
        "x2": rng.standard_normal((B, D, L)).astype(np.float32),
        "v": rng.standard_normal((B, D, L)).astype(np.float32),
        "h": (rng.standard_normal((D, L)) / math.sqrt(L) * 1e-5).astype(np.float32),
        "d_bias": rng.standard_normal(D).astype(np.float32),
    }
    out = kernel(**inputs)
    print(out.shape, out.dtype)
